# revision 8
# baseline (speedup 1.0000x reference)
"""ConfusionPenaltyLoss Trainium2 kernel.

Reference computation (for B=4096, T=128, C=37, L=8):
  positions = floor(linspace(0, T-1, L)) = [0,18,36,54,72,90,108,127]
  lp  = log_probs[:, positions, :]           # [B, L, C]
  tgt = targets.reshape(B, L)
  W[b,l,c] = mask[tgt[b,l], c]  where mask[g] = onehot(partner(g)) for the
             8 symmetric confusion pairs (else all-zero row)
  total = sum(W * exp(lp)) * 3.0 ; n = sum(W) ; out = total/n (0 if n==0)

Strategy: data-parallel over batch across 8 NeuronCores (512 batches/core).
Each core DMA-gathers only the 8 needed timesteps of its log_probs shard
(606 KB instead of 9.7 MB), computes exp + masked sums on-chip, and returns
per-partition partial (total, count). Host sums the 8x128 partials and
divides.

On-chip layout: 4096 (b,l) rows per core -> SBUF [128 part, 32 rows, 37 cls];
row i = b*8+l lives at partition i//32, row-slot i%32.
Partner lookup is arithmetic: P_enc = sum over ordered pairs (a->p) of
(tgt==a)*(p+1); W = (P_enc broadcast == iota(c+1)), so unpaired rows (enc 0)
match nothing.
"""

import numpy as np

NUM_CLASSES = 37
PENALTY_SCALE = 3.0
CONFUSION_PAIRS = [(1, 25), (2, 35), (5, 28), (8, 11), (13, 22), (6, 16), (9, 17), (3, 12)]
ORDERED_PAIRS = [(a, b) for a, b in CONFUSION_PAIRS] + [(b, a) for a, b in CONFUSION_PAIRS]

B, T, C, L = 4096, 128, 37, 8
N_CORES = 8
BS = B // N_CORES            # 512 batches per core
ROWS = BS * L                # 4096 (b,l) rows per core
JPP = ROWS // 128            # 32 rows per partition
POS_STEP = 18                # positions 0,18,...,108 then 127
N_UNIFORM = 7

_CACHE = {}


def _build_nc():
    import concourse.tile as tile
    from concourse import bacc, mybir

    f32 = mybir.dt.float32
    i32 = mybir.dt.int32
    Alu = mybir.AluOpType

    nc = bacc.Bacc("TRN2", target_bir_lowering=False, debug=False, num_devices=N_CORES)

    lp = nc.dram_tensor("lp", [BS, T, C], f32, kind="ExternalInput").ap()
    tg = nc.dram_tensor("tg", [ROWS], i32, kind="ExternalInput").ap()
    out = nc.dram_tensor("out", [128, 2], f32, kind="ExternalOutput").ap()

    with tile.TileContext(nc) as tc:
        with tc.tile_pool(name="main", bufs=1) as pool:
            LP = pool.tile([128, JPP * C], f32)
            TT = pool.tile([128, JPP], i32)

            # ---- DMA in ----
            # [BS,T,C] -> [128 part, 4 b_lo, T, C]
            lp4 = lp.rearrange("(ph bl) t c -> ph bl t c", bl=4)
            LPv = LP[:].rearrange("p (bl l c) -> p bl l c", bl=4, l=L)
            positions = [0, 18, 36, 54, 72, 90, 108, 127]
            # one 3-dim-AP DMA per gathered timestep (DMA APs max 3 dims)
            for l, t in enumerate(positions):
                nc.sync.dma_start(out=LPv[:, :, l, :], in_=lp4[:, :, t, :])
            nc.sync.dma_start(out=TT[:], in_=tg.rearrange("(p j) -> p j", p=128))

            # ---- partner encode: ACC = partner(tgt)+1, or 0 if unpaired ----
            ACC = pool.tile([128, JPP], i32)
            for k, (a, pb) in enumerate(ORDERED_PAIRS):
                if k == 0:
                    nc.vector.tensor_scalar(
                        out=ACC[:], in0=TT[:], scalar1=a, scalar2=pb + 1,
                        op0=Alu.is_equal, op1=Alu.mult,
                    )
                else:
                    TMP = pool.tile([128, JPP], i32, tag="tmp")
                    nc.vector.tensor_scalar(
                        out=TMP[:], in0=TT[:], scalar1=a, scalar2=pb + 1,
                        op0=Alu.is_equal, op1=Alu.mult,
                    )
                    nc.vector.tensor_tensor(
                        out=ACC[:], in0=ACC[:], in1=TMP[:], op=Alu.add
                    )

            # ---- iota of (c+1) repeated per row ----
            I37 = pool.tile([128, JPP * C], i32)
            nc.gpsimd.iota(
                I37[:], [[0, JPP], [1, C]], base=1, channel_multiplier=0
            )

            OUTT = pool.tile([128, 2], f32)

            # ---- NE = (P_enc != c+1) as f32 {0,1} ----
            NE = pool.tile([128, JPP * C], f32)
            I3 = I37[:].rearrange("p (j c) -> p j c", c=C)
            nc.vector.tensor_tensor(
                out=NE[:].rearrange("p (j c) -> p j c", c=C),
                in0=ACC[:].to_broadcast((128, JPP, C)),
                in1=I3,
                op=Alu.not_equal,
            )

            # ---- LPm = LP - 1e4 * NE  (log-domain mask-out) ----
            LPM = pool.tile([128, JPP * C], f32)
            nc.vector.scalar_tensor_tensor(
                out=LPM[:],
                in0=NE[:],
                scalar=-1.0e4,
                in1=LP[:],
                op0=Alu.mult,
                op1=Alu.add,
            )

            # ---- E = exp(LPm); accum gives per-partition masked total ----
            E = pool.tile([128, JPP * C], f32)
            nc.scalar.activation(
                out=E[:],
                in_=LPM[:],
                func=mybir.ActivationFunctionType.Exp,
                accum_out=OUTT[:, 0:1],
            )

            # ---- count = sum(P_enc > 0) per partition ----
            CNT = pool.tile([128, JPP], f32)
            nc.vector.tensor_scalar(
                out=CNT[:], in0=ACC[:], scalar1=0, scalar2=None, op0=Alu.is_gt
            )
            nc.vector.tensor_reduce(
                out=OUTT[:, 1:2], in_=CNT[:], axis=mybir.AxisListType.X, op=Alu.add
            )

            nc.sync.dma_start(out=out, in_=OUTT[:])

    nc.compile()
    return nc


def _get_nc():
    if "nc" not in _CACHE:
        _CACHE["nc"] = _build_nc()
    return _CACHE["nc"]


def _shard_inputs(log_probs, targets):
    lp = np.ascontiguousarray(np.asarray(log_probs, dtype=np.float32))
    tg = np.ascontiguousarray(np.asarray(targets).astype(np.int32))
    in_maps = []
    for i in range(N_CORES):
        in_maps.append(
            {
                "lp": lp[i * BS : (i + 1) * BS],
                "tg": tg[i * ROWS : (i + 1) * ROWS],
            }
        )
    return in_maps


def _combine(results):
    tot = 0.0
    cnt = 0.0
    for r in results:
        o = np.asarray(r["out"], dtype=np.float64)
        tot += o[:, 0].sum()
        cnt += o[:, 1].sum()
    if cnt > 0:
        return np.array(PENALTY_SCALE * tot / cnt, dtype=np.float32)
    return np.array(0.0, dtype=np.float32)


def kernel(log_probs, targets, target_lengths, **_kwargs):
    from concourse.bass_utils import run_bass_kernel_spmd

    nc = _get_nc()
    in_maps = _shard_inputs(log_probs, targets)
    res = run_bass_kernel_spmd(
        nc, in_maps, list(range(N_CORES)), **_CACHE.get("run_kwargs", {})
    )
    _CACHE["last_result"] = res
    return _combine(res.results)


# revision 14
# speedup vs baseline: 1.1503x; 1.1503x over previous
"""ConfusionPenaltyLoss Trainium2 kernel.

Reference computation (for B=4096, T=128, C=37, L=8):
  positions = floor(linspace(0, T-1, L)) = [0,18,36,54,72,90,108,127]
  lp  = log_probs[:, positions, :]           # [B, L, C]
  tgt = targets.reshape(B, L)
  W[b,l,c] = mask[tgt[b,l], c]  where mask[g] = onehot(partner(g)) for the
             8 symmetric confusion pairs (else all-zero row)
  total = sum(W * exp(lp)) * 3.0 ; n = sum(W) ; out = total/n (0 if n==0)

Strategy: data-parallel over batch across 8 NeuronCores (512 batches/core).
Each core DMA-gathers only the 8 needed timesteps of its log_probs shard
(606 KB instead of 9.7 MB), computes exp + masked sums on-chip, and returns
per-partition partial (total, count). Host sums the 8x128 partials and
divides.

On-chip layout: 4096 (b,l) rows per core -> SBUF [128 part, 32 rows, 37 cls];
row i = b*8+l lives at partition i//32, row-slot i%32.
Partner lookup is arithmetic: P_enc = sum over ordered pairs (a->p) of
(tgt==a)*(p+1); W = (P_enc broadcast == iota(c+1)), so unpaired rows (enc 0)
match nothing.
"""

import numpy as np

NUM_CLASSES = 37
PENALTY_SCALE = 3.0
CONFUSION_PAIRS = [(1, 25), (2, 35), (5, 28), (8, 11), (13, 22), (6, 16), (9, 17), (3, 12)]
ORDERED_PAIRS = [(a, b) for a, b in CONFUSION_PAIRS] + [(b, a) for a, b in CONFUSION_PAIRS]

B, T, C, L = 4096, 128, 37, 8
N_CORES = 8
BS = B // N_CORES            # 512 batches per core
ROWS = BS * L                # 4096 (b,l) rows per core
JPP = ROWS // 128            # 32 rows per partition
POS_STEP = 18                # positions 0,18,...,108 then 127
N_UNIFORM = 7

_CACHE = {}


def _build_nc():
    import concourse.tile as tile
    from concourse import bacc, mybir

    f32 = mybir.dt.float32
    i32 = mybir.dt.int32
    Alu = mybir.AluOpType

    nc = bacc.Bacc("TRN2", target_bir_lowering=False, debug=False, num_devices=N_CORES)

    lp = nc.dram_tensor("lp", [BS, T, C], f32, kind="ExternalInput").ap()
    tg = nc.dram_tensor("tg", [ROWS], i32, kind="ExternalInput").ap()
    out = nc.dram_tensor("out", [128, 3], f32, kind="ExternalOutput").ap()

    with tile.TileContext(nc) as tc:
        with tc.tile_pool(name="main", bufs=1) as pool:
            LP = pool.tile([128, JPP * C], f32)
            TT = pool.tile([128, JPP], i32)

            # ---- DMA in ----
            # Targets FIRST so the partner-encode chain can run under the
            # (much longer) LP gather. LP DMAs split across both HWDGE
            # rings (sync=SP, scalar=ACT) -> two desc-gen FIFOs.
            nc.sync.dma_start(out=TT[:], in_=tg.rearrange("(p j) -> p j", p=128))
            # [BS,T,C] -> [128 part, 4 b_lo, T, C]
            lp4 = lp.rearrange("(ph bl) t c -> ph bl t c", bl=4)
            LPv = LP[:].rearrange("p (bl l c) -> p bl l c", bl=4, l=L)
            positions = [0, 18, 36, 54, 72, 90, 108, 127]
            # one 3-dim-AP DMA per gathered timestep (DMA APs max 3 dims)
            for l, t in enumerate(positions):
                eng = nc.sync if l < 4 else nc.scalar
                eng.dma_start(out=LPv[:, :, l, :], in_=lp4[:, :, t, :])

            # ---- iota of (c+1) repeated per row (constant; overlaps DMA) ----
            I37 = pool.tile([128, JPP * C], i32)
            nc.gpsimd.iota(
                I37[:], [[0, JPP], [1, C]], base=1, channel_multiplier=0
            )

            # ---- partner encode: ACC = partner(tgt)+1, or 0 if unpaired ----
            # Split across vector and gpsimd so it finishes well before the
            # LP gather drains.
            def encode(eng, dst, prs, tmptag):
                for k, (a, pb) in enumerate(prs):
                    if k == 0:
                        eng.tensor_scalar(
                            out=dst[:], in0=TT[:], scalar1=a, scalar2=pb + 1,
                            op0=Alu.is_equal, op1=Alu.mult,
                        )
                    else:
                        TMP = pool.tile([128, JPP], i32, tag=tmptag)
                        eng.tensor_scalar(
                            out=TMP[:], in0=TT[:], scalar1=a, scalar2=pb + 1,
                            op0=Alu.is_equal, op1=Alu.mult,
                        )
                        eng.tensor_tensor(
                            out=dst[:], in0=dst[:], in1=TMP[:], op=Alu.add
                        )

            ACCv = pool.tile([128, JPP], i32)
            ACCg = pool.tile([128, JPP], i32)
            encode(nc.vector, ACCv, ORDERED_PAIRS[:8], "tmpv")
            encode(nc.gpsimd, ACCg, ORDERED_PAIRS[8:], "tmpg")
            ACC = pool.tile([128, JPP], i32)
            nc.vector.tensor_tensor(
                out=ACC[:], in0=ACCv[:], in1=ACCg[:], op=Alu.add
            )

            OUTT = pool.tile([128, 3], f32)

            # ---- NE = (P_enc != c+1) as f32 {0,1}; overlaps LP DMA ----
            NE = pool.tile([128, JPP * C], f32)
            I3 = I37[:].rearrange("p (j c) -> p j c", c=C)
            nc.vector.tensor_tensor(
                out=NE[:].rearrange("p (j c) -> p j c", c=C),
                in0=ACC[:].to_broadcast((128, JPP, C)),
                in1=I3,
                op=Alu.not_equal,
            )

            # ---- count = sum(P_enc > 0) per partition (off critical path) ----
            CNT = pool.tile([128, JPP], f32)
            nc.vector.tensor_scalar(
                out=CNT[:], in0=ACC[:], scalar1=0, scalar2=None, op0=Alu.is_gt
            )
            nc.vector.tensor_reduce(
                out=OUTT[:, 2:3], in_=CNT[:], axis=mybir.AxisListType.X, op=Alu.add
            )

            # ---- two halves: LPm = LP - 1e4*NE, then exp with accum.
            # Half A (l=0..3, sync-ring DMAs) computes while half B's DMAs
            # (scalar ring) still drain.
            LPM = pool.tile([128, JPP * C], f32)
            E = pool.tile([128, JPP * C], f32)
            NEv = NE[:].rearrange("p (bl l c) -> p bl l c", bl=4, l=L)
            LPMv = LPM[:].rearrange("p (bl l c) -> p bl l c", bl=4, l=L)
            Ev = E[:].rearrange("p (bl l c) -> p bl l c", bl=4, l=L)
            for h in range(2):
                ls = slice(4 * h, 4 * h + 4)
                nc.vector.scalar_tensor_tensor(
                    out=LPMv[:, :, ls, :],
                    in0=NEv[:, :, ls, :],
                    scalar=-1.0e4,
                    in1=LPv[:, :, ls, :],
                    op0=Alu.mult,
                    op1=Alu.add,
                )
                nc.scalar.activation(
                    out=Ev[:, :, ls, :],
                    in_=LPMv[:, :, ls, :],
                    func=mybir.ActivationFunctionType.Exp,
                    accum_out=OUTT[:, h : h + 1],
                )

            nc.sync.dma_start(out=out, in_=OUTT[:])

    nc.compile()
    return nc


def _get_nc():
    if "nc" not in _CACHE:
        _CACHE["nc"] = _build_nc()
    return _CACHE["nc"]


def _shard_inputs(log_probs, targets):
    lp = np.ascontiguousarray(np.asarray(log_probs, dtype=np.float32))
    tg = np.ascontiguousarray(np.asarray(targets).astype(np.int32))
    in_maps = []
    for i in range(N_CORES):
        in_maps.append(
            {
                "lp": lp[i * BS : (i + 1) * BS],
                "tg": tg[i * ROWS : (i + 1) * ROWS],
            }
        )
    return in_maps


def _combine(results):
    tot = 0.0
    cnt = 0.0
    for r in results:
        o = np.asarray(r["out"], dtype=np.float64)
        tot += o[:, 0].sum() + o[:, 1].sum()
        cnt += o[:, 2].sum()
    if cnt > 0:
        return np.array(PENALTY_SCALE * tot / cnt, dtype=np.float32)
    return np.array(0.0, dtype=np.float32)


def kernel(log_probs, targets, target_lengths, **_kwargs):
    from concourse.bass_utils import run_bass_kernel_spmd

    nc = _get_nc()
    in_maps = _shard_inputs(log_probs, targets)
    res = run_bass_kernel_spmd(
        nc, in_maps, list(range(N_CORES)), **_CACHE.get("run_kwargs", {})
    )
    _CACHE["last_result"] = res
    return _combine(res.results)


# revision 17
# speedup vs baseline: 1.4810x; 1.2875x over previous
"""ConfusionPenaltyLoss Trainium2 kernel.

Reference computation (for B=4096, T=128, C=37, L=8):
  positions = floor(linspace(0, T-1, L)) = [0,18,36,54,72,90,108,127]
  lp  = log_probs[:, positions, :]           # [B, L, C]
  tgt = targets.reshape(B, L)
  W[b,l,c] = mask[tgt[b,l], c]  where mask[g] = onehot(partner(g)) for the
             8 symmetric confusion pairs (else all-zero row)
  total = sum(W * exp(lp)) * 3.0 ; n = sum(W) ; out = total/n (0 if n==0)

Strategy: data-parallel over batch across 8 NeuronCores (512 batches/core).
Each core DMA-gathers only the 8 needed timesteps of its log_probs shard
(606 KB instead of 9.7 MB), computes exp + masked sums on-chip, and returns
per-partition partial (total, count). Host sums the 8x128 partials and
divides.

On-chip layout: 4096 (b,l) rows per core -> SBUF [128 part, 32 rows, 37 cls];
row i = b*8+l lives at partition i//32, row-slot i%32.
Partner lookup is arithmetic: P_enc = sum over ordered pairs (a->p) of
(tgt==a)*(p+1); W = (P_enc broadcast == iota(c+1)), so unpaired rows (enc 0)
match nothing.
"""

import numpy as np

NUM_CLASSES = 37
PENALTY_SCALE = 3.0
CONFUSION_PAIRS = [(1, 25), (2, 35), (5, 28), (8, 11), (13, 22), (6, 16), (9, 17), (3, 12)]
ORDERED_PAIRS = [(a, b) for a, b in CONFUSION_PAIRS] + [(b, a) for a, b in CONFUSION_PAIRS]

B, T, C, L = 4096, 128, 37, 8
N_CORES = 8
BS = B // N_CORES            # 512 batches per core
ROWS = BS * L                # 4096 (b,l) rows per core
JPP = ROWS // 128            # 32 rows per partition
POS_STEP = 18                # positions 0,18,...,108 then 127
N_UNIFORM = 7

_CACHE = {}


def _build_nc():
    import concourse.tile as tile
    from concourse import bacc, mybir

    f32 = mybir.dt.float32
    i32 = mybir.dt.int32
    Alu = mybir.AluOpType

    nc = bacc.Bacc("TRN2", target_bir_lowering=False, debug=False, num_devices=N_CORES)

    lp = nc.dram_tensor("lp", [BS, T, C], f32, kind="ExternalInput").ap()
    tgc = nc.dram_tensor("tgc", [128, 64], i32, kind="ExternalInput").ap()
    out = nc.dram_tensor("out", [128, 3], f32, kind="ExternalOutput").ap()

    NPAIR = len(ORDERED_PAIRS)  # 16

    with tile.TileContext(nc) as tc:
        with tc.tile_pool(name="main", bufs=1) as pool:
            LP = pool.tile([128, JPP * C], f32)
            CONSTS = pool.tile([128, 64], i32)

            # ---- DMA in ----
            # Small (targets + pair tables) FIRST so the partner-encode can
            # run under the (much longer) LP gather. LP DMAs split across
            # both HWDGE rings (sync=SP, scalar=ACT) -> two desc-gen FIFOs.
            nc.sync.dma_start(out=CONSTS[:], in_=tgc)
            # [BS,T,C] -> [128 part, 4 b_lo, T, C]
            lp4 = lp.rearrange("(ph bl) t c -> ph bl t c", bl=4)
            LPv = LP[:].rearrange("p (bl l c) -> p bl l c", bl=4, l=L)
            positions = [0, 18, 36, 54, 72, 90, 108, 127]
            # one 3-dim-AP DMA per gathered timestep (DMA APs max 3 dims)
            for l, t in enumerate(positions):
                eng = nc.sync if l < 4 else nc.scalar
                eng.dma_start(out=LPv[:, :, l, :], in_=lp4[:, :, t, :])

            # ---- iota of (c+1) repeated per row (constant; overlaps DMA) ----
            I37 = pool.tile([128, JPP * C], i32)
            nc.gpsimd.iota(
                I37[:], [[0, JPP], [1, C]], base=1, channel_multiplier=0
            )

            # ---- partner encode via 3 wide ops over a pairs axis ----
            # CMP[p,j,k] = (tgt[p,j] == a_k); P_enc = sum_k CMP*enc_k
            # (enc_k = partner(a_k)+1, so unpaired targets encode to 0).
            TT = CONSTS[:, 0:JPP]
            AVEC = CONSTS[:, 32:48]
            ENCV = CONSTS[:, 48:64]

            def bcast_mid(ap2d, n):
                return ap2d.rearrange("p (one k) -> p one k", one=1).broadcast_to(
                    (128, n, NPAIR)
                )

            CMP = pool.tile([128, JPP * NPAIR], i32)
            CMP3 = CMP[:].rearrange("p (j k) -> p j k", k=NPAIR)
            nc.vector.tensor_tensor(
                out=CMP3,
                in0=TT.to_broadcast((128, JPP, NPAIR)),
                in1=bcast_mid(AVEC, JPP),
                op=Alu.is_equal,
            )
            PV = pool.tile([128, JPP * NPAIR], i32)
            PV3 = PV[:].rearrange("p (j k) -> p j k", k=NPAIR)
            nc.vector.tensor_tensor(
                out=PV3, in0=CMP3, in1=bcast_mid(ENCV, JPP), op=Alu.mult
            )
            ACC = pool.tile([128, JPP], i32)
            with nc.allow_low_precision(reason="exact small-int reduce"):
                nc.vector.tensor_reduce(
                    out=ACC[:], in_=PV3, axis=mybir.AxisListType.X, op=Alu.add
                )

            OUTT = pool.tile([128, 3], f32)

            # ---- count = sum(CMP) per partition (off critical path) ----
            nc.vector.tensor_reduce(
                out=OUTT[:, 2:3], in_=CMP3, axis=mybir.AxisListType.XY, op=Alu.add
            )

            # ---- NE = (P_enc != c+1) as f32 {0,1}; overlaps LP DMA ----
            NE = pool.tile([128, JPP * C], f32)
            I3 = I37[:].rearrange("p (j c) -> p j c", c=C)
            nc.vector.tensor_tensor(
                out=NE[:].rearrange("p (j c) -> p j c", c=C),
                in0=ACC[:].to_broadcast((128, JPP, C)),
                in1=I3,
                op=Alu.not_equal,
            )

            # ---- two halves: LPm = LP - 1e4*NE, then exp with accum.
            # Half A (l=0..3, sync-ring DMAs) computes while half B's DMAs
            # (scalar ring) still drain.
            LPM = pool.tile([128, JPP * C], f32)
            E = pool.tile([128, JPP * C], f32)
            NEv = NE[:].rearrange("p (bl l c) -> p bl l c", bl=4, l=L)
            LPMv = LPM[:].rearrange("p (bl l c) -> p bl l c", bl=4, l=L)
            Ev = E[:].rearrange("p (bl l c) -> p bl l c", bl=4, l=L)
            for h in range(2):
                ls = slice(4 * h, 4 * h + 4)
                nc.vector.scalar_tensor_tensor(
                    out=LPMv[:, :, ls, :],
                    in0=NEv[:, :, ls, :],
                    scalar=-1.0e4,
                    in1=LPv[:, :, ls, :],
                    op0=Alu.mult,
                    op1=Alu.add,
                )
                nc.scalar.activation(
                    out=Ev[:, :, ls, :],
                    in_=LPMv[:, :, ls, :],
                    func=mybir.ActivationFunctionType.Exp,
                    accum_out=OUTT[:, h : h + 1],
                )

            nc.sync.dma_start(out=out, in_=OUTT[:])

    nc.compile()
    return nc


def _get_nc():
    if "nc" not in _CACHE:
        _CACHE["nc"] = _build_nc()
    return _CACHE["nc"]


def _shard_inputs(log_probs, targets):
    lp = np.ascontiguousarray(np.asarray(log_probs, dtype=np.float32))
    tg = np.ascontiguousarray(np.asarray(targets).astype(np.int32))
    avec = np.array([a for a, _ in ORDERED_PAIRS], dtype=np.int32)
    encv = np.array([b + 1 for _, b in ORDERED_PAIRS], dtype=np.int32)
    in_maps = []
    for i in range(N_CORES):
        tgc = np.empty((128, 64), dtype=np.int32)
        tgc[:, :32] = tg[i * ROWS : (i + 1) * ROWS].reshape(128, 32)
        tgc[:, 32:48] = avec
        tgc[:, 48:64] = encv
        in_maps.append(
            {
                "lp": lp[i * BS : (i + 1) * BS],
                "tgc": tgc,
            }
        )
    return in_maps


def _combine(results):
    tot = 0.0
    cnt = 0.0
    for r in results:
        o = np.asarray(r["out"], dtype=np.float64)
        tot += o[:, 0].sum() + o[:, 1].sum()
        cnt += o[:, 2].sum()
    if cnt > 0:
        return np.array(PENALTY_SCALE * tot / cnt, dtype=np.float32)
    return np.array(0.0, dtype=np.float32)


def kernel(log_probs, targets, target_lengths, **_kwargs):
    from concourse.bass_utils import run_bass_kernel_spmd

    nc = _get_nc()
    in_maps = _shard_inputs(log_probs, targets)
    res = run_bass_kernel_spmd(
        nc, in_maps, list(range(N_CORES)), **_CACHE.get("run_kwargs", {})
    )
    _CACHE["last_result"] = res
    return _combine(res.results)


# revision 21
# speedup vs baseline: 1.4983x; 1.0117x over previous
"""ConfusionPenaltyLoss Trainium2 kernel.

Reference computation (for B=4096, T=128, C=37, L=8):
  positions = floor(linspace(0, T-1, L)) = [0,18,36,54,72,90,108,127]
  lp  = log_probs[:, positions, :]           # [B, L, C]
  tgt = targets.reshape(B, L)
  W[b,l,c] = mask[tgt[b,l], c]  where mask[g] = onehot(partner(g)) for the
             8 symmetric confusion pairs (else all-zero row)
  total = sum(W * exp(lp)) * 3.0 ; n = sum(W) ; out = total/n (0 if n==0)

Strategy: data-parallel over batch across 8 NeuronCores (512 batches/core).
Each core DMA-gathers only the 8 needed timesteps of its log_probs shard
(606 KB instead of 9.7 MB), computes exp + masked sums on-chip, and returns
per-partition partial (total, count). Host sums the 8x128 partials and
divides.

On-chip layout: 4096 (b,l) rows per core -> SBUF [128 part, 32 rows, 37 cls];
row i = b*8+l lives at partition i//32, row-slot i%32.
Partner lookup is arithmetic: P_enc = sum over ordered pairs (a->p) of
(tgt==a)*(p+1); W = (P_enc broadcast == iota(c+1)), so unpaired rows (enc 0)
match nothing.
"""

import numpy as np

NUM_CLASSES = 37
PENALTY_SCALE = 3.0
CONFUSION_PAIRS = [(1, 25), (2, 35), (5, 28), (8, 11), (13, 22), (6, 16), (9, 17), (3, 12)]
ORDERED_PAIRS = [(a, b) for a, b in CONFUSION_PAIRS] + [(b, a) for a, b in CONFUSION_PAIRS]

B, T, C, L = 4096, 128, 37, 8
N_CORES = 8
BS = B // N_CORES            # 512 batches per core
ROWS = BS * L                # 4096 (b,l) rows per core
JPP = ROWS // 128            # 32 rows per partition
POS_STEP = 18                # positions 0,18,...,108 then 127
N_UNIFORM = 7

_CACHE = {}


def _build_nc():
    from contextlib import ExitStack

    from concourse import bacc, mybir

    f32 = mybir.dt.float32
    i32 = mybir.dt.int32
    Alu = mybir.AluOpType

    nc = bacc.Bacc("TRN2", target_bir_lowering=False, debug=False, num_devices=N_CORES)

    lp = nc.dram_tensor("lp", [BS, T, C], f32, kind="ExternalInput").ap()
    tgc = nc.dram_tensor("tgc", [128, 64], i32, kind="ExternalInput").ap()
    out = nc.dram_tensor("out", [128, 3], f32, kind="ExternalOutput").ap()

    NPAIR = len(ORDERED_PAIRS)  # 16
    positions = [0, 18, 36, 54, 72, 90, 108, 127]

    with ExitStack() as ctx:
        sb = lambda name, shape, dt: ctx.enter_context(
            nc.sbuf_tensor(name, shape, dt)
        ).ap()
        LP = sb("LP", [128, JPP * C], f32)
        CONSTS = sb("CONSTS", [128, 64], i32)
        I37 = sb("I37", [128, JPP * C], i32)
        CMP = sb("CMP", [128, JPP * NPAIR], i32)
        PV = sb("PV", [128, JPP * NPAIR], i32)
        ACC = sb("ACC", [128, JPP], i32)
        NE = sb("NE", [128, JPP * C], f32)
        LPM = sb("LPM", [128, JPP * C], f32)
        E = sb("E", [128, JPP * C], f32)
        OUTT = sb("OUTT", [128, 3], f32)

        s_small = ctx.enter_context(nc.semaphore("s_small"))
        s_lpa = ctx.enter_context(nc.semaphore("s_lpa"))
        s_lpb = ctx.enter_context(nc.semaphore("s_lpb"))
        s_iota = ctx.enter_context(nc.semaphore("s_iota"))
        s_lpm = ctx.enter_context(nc.semaphore("s_lpm"))
        s_exp = ctx.enter_context(nc.semaphore("s_exp"))
        s_out = ctx.enter_context(nc.semaphore("s_out"))
        s_outdma = ctx.enter_context(nc.semaphore("s_outdma"))
        s_v = ctx.enter_context(nc.semaphore("s_v"))

        lp4 = lp.rearrange("(ph bl) t c -> ph bl t c", bl=4)
        LPv = LP.rearrange("p (bl l c) -> p bl l c", bl=4, l=L)
        CMP3 = CMP.rearrange("p (j k) -> p j k", k=NPAIR)
        PV3 = PV.rearrange("p (j k) -> p j k", k=NPAIR)
        I3 = I37.rearrange("p (j c) -> p j c", c=C)
        NE3 = NE.rearrange("p (j c) -> p j c", c=C)
        E3 = E.rearrange("p (j c) -> p j c", c=C)
        NEv = NE.rearrange("p (bl l c) -> p bl l c", bl=4, l=L)
        LPMv = LPM.rearrange("p (bl l c) -> p bl l c", bl=4, l=L)
        Ev = E.rearrange("p (bl l c) -> p bl l c", bl=4, l=L)
        TT = CONSTS[:, 0:JPP]
        AVEC = CONSTS[:, 32:48]
        ENCV = CONSTS[:, 48:64]

        def bcast_mid(ap2d, n):
            return ap2d.rearrange("p (one k) -> p one k", one=1).broadcast_to(
                (128, n, NPAIR)
            )

        with nc.Block() as block:

            @block.sync
            def _(sync):
                # small consts first, then half the LP gather on this ring
                sync.dma_start(out=CONSTS[:], in_=tgc).then_inc(s_small, 16)
                for l in range(4):
                    sync.dma_start(
                        out=LPv[:, :, l, :], in_=lp4[:, :, positions[l], :]
                    ).then_inc(s_lpa, 16)
                sync.wait_ge(s_out, 3)
                sync.dma_start(out=out, in_=OUTT[:]).then_inc(s_outdma, 16)
                sync.wait_ge(s_outdma, 16)

            @block.scalar
            def _(scalar):
                # other half of the LP gather on the ACT HWDGE ring
                for l in range(4, 8):
                    scalar.dma_start(
                        out=LPv[:, :, l, :], in_=lp4[:, :, positions[l], :]
                    ).then_inc(s_lpb, 16)
                scalar.wait_ge(s_lpm, 1)
                scalar.activation(
                    out=Ev[:, :, 0:4, :],
                    in_=LPMv[:, :, 0:4, :],
                    func=mybir.ActivationFunctionType.Exp,
                ).then_inc(s_exp, 1)
                scalar.wait_ge(s_lpm, 2)
                scalar.activation(
                    out=Ev[:, :, 4:8, :],
                    in_=LPMv[:, :, 4:8, :],
                    func=mybir.ActivationFunctionType.Exp,
                ).then_inc(s_exp, 1)

            @block.gpsimd
            def _(gpsimd):
                gpsimd.iota(
                    I37[:], [[0, JPP], [1, C]], base=1, channel_multiplier=0
                ).then_inc(s_iota, 1)

            @block.vector
            def _(vector):
                # s_v: same-engine pipeline chain (DVE write->read hazard)
                # partner encode: CMP[p,j,k] = (tgt==a_k); P_enc = sum_k CMP*enc_k
                vector.wait_ge(s_small, 16)
                vector.tensor_tensor(
                    out=CMP3,
                    in0=TT.to_broadcast((128, JPP, NPAIR)),
                    in1=bcast_mid(AVEC, JPP),
                    op=Alu.is_equal,
                ).then_inc(s_v, 1)
                vector.wait_ge(s_v, 1)
                vector.tensor_tensor(
                    out=PV3, in0=CMP3, in1=bcast_mid(ENCV, JPP), op=Alu.mult
                ).then_inc(s_v, 1)
                vector.wait_ge(s_v, 2)
                with nc.allow_low_precision(reason="exact small-int reduce"):
                    vector.tensor_reduce(
                        out=ACC[:], in_=PV3, axis=mybir.AxisListType.X, op=Alu.add
                    ).then_inc(s_v, 1)
                # NE = (P_enc != c+1) as f32 {0,1}
                vector.wait_ge(s_v, 3)
                vector.wait_ge(s_iota, 1)
                vector.tensor_tensor(
                    out=NE3,
                    in0=ACC[:].to_broadcast((128, JPP, C)),
                    in1=I3,
                    op=Alu.not_equal,
                ).then_inc(s_v, 1)
                # count = sum(CMP): off critical path, after NE
                vector.tensor_reduce(
                    out=OUTT[:, 2:3], in_=CMP3, axis=mybir.AxisListType.XY, op=Alu.add
                ).then_inc(s_out, 1)
                # LPm = LP - 1e4*NE per half as each ring's gather lands
                vector.wait_ge(s_v, 4)
                vector.wait_ge(s_lpa, 64)
                vector.scalar_tensor_tensor(
                    out=LPMv[:, :, 0:4, :],
                    in0=NEv[:, :, 0:4, :],
                    scalar=-1.0e4,
                    in1=LPv[:, :, 0:4, :],
                    op0=Alu.mult,
                    op1=Alu.add,
                ).then_inc(s_lpm, 1)
                vector.wait_ge(s_lpb, 64)
                vector.scalar_tensor_tensor(
                    out=LPMv[:, :, 4:8, :],
                    in0=NEv[:, :, 4:8, :],
                    scalar=-1.0e4,
                    in1=LPv[:, :, 4:8, :],
                    op0=Alu.mult,
                    op1=Alu.add,
                ).then_inc(s_lpm, 1)
                # masked totals per half
                vector.wait_ge(s_exp, 1)
                vector.tensor_reduce(
                    out=OUTT[:, 0:1],
                    in_=Ev[:, :, 0:4, :],
                    axis=mybir.AxisListType.XYZ,
                    op=Alu.add,
                ).then_inc(s_out, 1)
                vector.wait_ge(s_exp, 2)
                vector.tensor_reduce(
                    out=OUTT[:, 1:2],
                    in_=Ev[:, :, 4:8, :],
                    axis=mybir.AxisListType.XYZ,
                    op=Alu.add,
                ).then_inc(s_out, 1)

    nc.compile()
    return nc


def _get_nc():
    if "nc" not in _CACHE:
        _CACHE["nc"] = _build_nc()
    return _CACHE["nc"]


def _shard_inputs(log_probs, targets):
    lp = np.ascontiguousarray(np.asarray(log_probs, dtype=np.float32))
    tg = np.ascontiguousarray(np.asarray(targets).astype(np.int32))
    avec = np.array([a for a, _ in ORDERED_PAIRS], dtype=np.int32)
    encv = np.array([b + 1 for _, b in ORDERED_PAIRS], dtype=np.int32)
    in_maps = []
    for i in range(N_CORES):
        tgc = np.empty((128, 64), dtype=np.int32)
        tgc[:, :32] = tg[i * ROWS : (i + 1) * ROWS].reshape(128, 32)
        tgc[:, 32:48] = avec
        tgc[:, 48:64] = encv
        in_maps.append(
            {
                "lp": lp[i * BS : (i + 1) * BS],
                "tgc": tgc,
            }
        )
    return in_maps


def _combine(results):
    tot = 0.0
    cnt = 0.0
    for r in results:
        o = np.asarray(r["out"], dtype=np.float64)
        tot += o[:, 0].sum() + o[:, 1].sum()
        cnt += o[:, 2].sum()
    if cnt > 0:
        return np.array(PENALTY_SCALE * tot / cnt, dtype=np.float32)
    return np.array(0.0, dtype=np.float32)


def kernel(log_probs, targets, target_lengths, **_kwargs):
    from concourse.bass_utils import run_bass_kernel_spmd

    nc = _get_nc()
    in_maps = _shard_inputs(log_probs, targets)
    res = run_bass_kernel_spmd(
        nc, in_maps, list(range(N_CORES)), **_CACHE.get("run_kwargs", {})
    )
    _CACHE["last_result"] = res
    return _combine(res.results)


# revision 25
# speedup vs baseline: 1.5570x; 1.0392x over previous
"""ConfusionPenaltyLoss Trainium2 kernel.

Reference computation (for B=4096, T=128, C=37, L=8):
  positions = floor(linspace(0, T-1, L)) = [0,18,36,54,72,90,108,127]
  lp  = log_probs[:, positions, :]           # [B, L, C]
  tgt = targets.reshape(B, L)
  W[b,l,c] = mask[tgt[b,l], c]  where mask[g] = onehot(partner(g)) for the
             8 symmetric confusion pairs (else all-zero row)
  total = sum(W * exp(lp)) * 3.0 ; n = sum(W) ; out = total/n (0 if n==0)

Strategy: data-parallel over batch across 8 NeuronCores (512 batches/core).
Each core DMA-gathers only the 8 needed timesteps of its log_probs shard
(606 KB instead of 9.7 MB), computes exp + masked sums on-chip, and returns
per-partition partial (total, count). Host sums the 8x128 partials and
divides.

On-chip layout: 4096 (b,l) rows per core -> SBUF [128 part, 32 rows, 37 cls];
row i = b*8+l lives at partition i//32, row-slot i%32.
Partner lookup is arithmetic: P_enc = sum over ordered pairs (a->p) of
(tgt==a)*(p+1); W = (P_enc broadcast == iota(c+1)), so unpaired rows (enc 0)
match nothing.
"""

import numpy as np

NUM_CLASSES = 37
PENALTY_SCALE = 3.0
CONFUSION_PAIRS = [(1, 25), (2, 35), (5, 28), (8, 11), (13, 22), (6, 16), (9, 17), (3, 12)]
ORDERED_PAIRS = [(a, b) for a, b in CONFUSION_PAIRS] + [(b, a) for a, b in CONFUSION_PAIRS]

B, T, C, L = 4096, 128, 37, 8
N_CORES = 8
BS = B // N_CORES            # 512 batches per core
ROWS = BS * L                # 4096 (b,l) rows per core
JPP = ROWS // 128            # 32 rows per partition
POS_STEP = 18                # positions 0,18,...,108 then 127
N_UNIFORM = 7

_CACHE = {}


def _build_nc():
    from contextlib import ExitStack

    from concourse import bacc, mybir

    f32 = mybir.dt.float32
    i32 = mybir.dt.int32
    Alu = mybir.AluOpType

    nc = bacc.Bacc("TRN2", target_bir_lowering=False, debug=False, num_devices=N_CORES)

    lp = nc.dram_tensor("lp", [BS, T, C], f32, kind="ExternalInput").ap()
    tgc = nc.dram_tensor("tgc", [128, 64], i32, kind="ExternalInput").ap()
    out = nc.dram_tensor("out", [128, 3], f32, kind="ExternalOutput").ap()

    NPAIR = len(ORDERED_PAIRS)  # 16
    positions = [0, 18, 36, 54, 72, 90, 108, 127]

    with ExitStack() as ctx:
        sb = lambda name, shape, dt: ctx.enter_context(
            nc.sbuf_tensor(name, shape, dt)
        ).ap()
        LP = sb("LP", [128, JPP * C], f32)
        CONSTS = sb("CONSTS", [128, 64], i32)
        I37 = sb("I37", [128, JPP * C], i32)
        CMP = sb("CMP", [128, JPP * NPAIR], i32)
        PV = sb("PV", [128, JPP * NPAIR], i32)
        ACC = sb("ACC", [128, JPP], i32)
        NE = sb("NE", [128, JPP * C], f32)
        LPM = sb("LPM", [128, JPP * C], f32)
        E = sb("E", [128, JPP * C], f32)
        OUTT = sb("OUTT", [128, 3], f32)

        s_small = ctx.enter_context(nc.semaphore("s_small"))
        s_lpa = ctx.enter_context(nc.semaphore("s_lpa"))
        s_lpb = ctx.enter_context(nc.semaphore("s_lpb"))
        s_iota = ctx.enter_context(nc.semaphore("s_iota"))
        s_lpm = ctx.enter_context(nc.semaphore("s_lpm"))
        s_exp = ctx.enter_context(nc.semaphore("s_exp"))
        s_out = ctx.enter_context(nc.semaphore("s_out"))
        s_outdma = ctx.enter_context(nc.semaphore("s_outdma"))
        s_v = ctx.enter_context(nc.semaphore("s_v"))

        lp4 = lp.rearrange("(ph bl) t c -> ph bl t c", bl=4)
        LPv = LP.rearrange("p (bl l c) -> p bl l c", bl=4, l=L)
        CMP3 = CMP.rearrange("p (j k) -> p j k", k=NPAIR)
        PV3 = PV.rearrange("p (j k) -> p j k", k=NPAIR)
        I3 = I37.rearrange("p (j c) -> p j c", c=C)
        NE3 = NE.rearrange("p (j c) -> p j c", c=C)
        E3 = E.rearrange("p (j c) -> p j c", c=C)
        NEv = NE.rearrange("p (bl l c) -> p bl l c", bl=4, l=L)
        LPMv = LPM.rearrange("p (bl l c) -> p bl l c", bl=4, l=L)
        Ev = E.rearrange("p (bl l c) -> p bl l c", bl=4, l=L)
        TT = CONSTS[:, 0:JPP]
        AVEC = CONSTS[:, 32:48]
        ENCV = CONSTS[:, 48:64]

        def bcast_mid(ap2d, n):
            return ap2d.rearrange("p (one k) -> p one k", one=1).broadcast_to(
                (128, n, NPAIR)
            )

        with nc.Block() as block:

            @block.sync
            def _(sync):
                for l in range(4):
                    sync.dma_start(
                        out=LPv[:, :, l, :], in_=lp4[:, :, positions[l], :]
                    ).then_inc(s_lpa, 16)
                sync.wait_ge(s_out, 3)
                # no receipt wait: NEFF teardown (sem sweep + end barrier)
                # far outlasts the 1.5KB write
                sync.dma_start(out=out, in_=OUTT[:]).then_inc(s_outdma, 16)

            @block.scalar
            def _(scalar):
                # other half of the LP gather on the ACT HWDGE ring
                for l in range(4, 8):
                    scalar.dma_start(
                        out=LPv[:, :, l, :], in_=lp4[:, :, positions[l], :]
                    ).then_inc(s_lpb, 16)
                scalar.wait_ge(s_lpm, 1)
                scalar.activation(
                    out=Ev[:, :, 0:4, :],
                    in_=LPMv[:, :, 0:4, :],
                    func=mybir.ActivationFunctionType.Exp,
                ).then_inc(s_exp, 1)
                scalar.wait_ge(s_lpm, 2)
                scalar.activation(
                    out=Ev[:, :, 4:8, :],
                    in_=LPMv[:, :, 4:8, :],
                    func=mybir.ActivationFunctionType.Exp,
                ).then_inc(s_exp, 1)

            @block.gpsimd
            def _(gpsimd):
                # consts via SWDGE so the HWDGE rings carry only the LP gather
                gpsimd.dma_start(out=CONSTS[:], in_=tgc).then_inc(s_small, 16)
                gpsimd.iota(
                    I37[:], [[0, JPP], [1, C]], base=1, channel_multiplier=0
                ).then_inc(s_iota, 1)

            @block.vector
            def _(vector):
                # s_v: same-engine pipeline chain (DVE write->read hazard)
                # partner encode: CMP[p,j,k] = (tgt==a_k); P_enc = sum_k CMP*enc_k
                vector.wait_ge(s_small, 16)
                vector.tensor_tensor(
                    out=CMP3,
                    in0=TT.to_broadcast((128, JPP, NPAIR)),
                    in1=bcast_mid(AVEC, JPP),
                    op=Alu.is_equal,
                ).then_inc(s_v, 1)
                vector.wait_ge(s_v, 1)
                vector.tensor_tensor(
                    out=PV3, in0=CMP3, in1=bcast_mid(ENCV, JPP), op=Alu.mult
                ).then_inc(s_v, 1)
                vector.wait_ge(s_v, 2)
                with nc.allow_low_precision(reason="exact small-int reduce"):
                    vector.tensor_reduce(
                        out=ACC[:], in_=PV3, axis=mybir.AxisListType.X, op=Alu.add
                    ).then_inc(s_v, 1)
                # NE = (P_enc != c+1) as f32 {0,1}, split by l-halves so the
                # first mask-apply starts as soon as half A is comparable
                ACCb = ACC[:].rearrange("p (bl l) -> p bl l", l=L)
                I4 = I37.rearrange("p (bl l c) -> p bl l c", bl=4, l=L)
                vector.wait_ge(s_v, 3)
                vector.wait_ge(s_iota, 1)
                for h in range(2):
                    ls = slice(4 * h, 4 * h + 4)
                    vector.tensor_tensor(
                        out=NEv[:, :, ls, :],
                        in0=ACCb[:, :, ls].to_broadcast((128, 4, 4, C)),
                        in1=I4[:, :, ls, :],
                        op=Alu.not_equal,
                    ).then_inc(s_v, 1)
                for h in range(2):
                    ls = slice(4 * h, 4 * h + 4)
                    # LPm = LP - 1e4*NE once this half's gather has landed
                    vector.wait_ge(s_v, 4 + h)
                    vector.wait_ge(s_lpa if h == 0 else s_lpb, 64)
                    vector.scalar_tensor_tensor(
                        out=LPMv[:, :, ls, :],
                        in0=NEv[:, :, ls, :],
                        scalar=-1.0e4,
                        in1=LPv[:, :, ls, :],
                        op0=Alu.mult,
                        op1=Alu.add,
                    ).then_inc(s_lpm, 1)
                # count = sum(CMP): fills the gap while exp runs on ACT
                vector.tensor_reduce(
                    out=OUTT[:, 2:3], in_=CMP3, axis=mybir.AxisListType.XY, op=Alu.add
                ).then_inc(s_out, 1)
                # masked totals per half
                vector.wait_ge(s_exp, 1)
                vector.tensor_reduce(
                    out=OUTT[:, 0:1],
                    in_=Ev[:, :, 0:4, :],
                    axis=mybir.AxisListType.XYZ,
                    op=Alu.add,
                ).then_inc(s_out, 1)
                vector.wait_ge(s_exp, 2)
                vector.tensor_reduce(
                    out=OUTT[:, 1:2],
                    in_=Ev[:, :, 4:8, :],
                    axis=mybir.AxisListType.XYZ,
                    op=Alu.add,
                ).then_inc(s_out, 1)

    nc.compile()
    return nc


def _get_nc():
    if "nc" not in _CACHE:
        _CACHE["nc"] = _build_nc()
    return _CACHE["nc"]


def _shard_inputs(log_probs, targets):
    lp = np.ascontiguousarray(np.asarray(log_probs, dtype=np.float32))
    tg = np.ascontiguousarray(np.asarray(targets).astype(np.int32))
    avec = np.array([a for a, _ in ORDERED_PAIRS], dtype=np.int32)
    encv = np.array([b + 1 for _, b in ORDERED_PAIRS], dtype=np.int32)
    in_maps = []
    for i in range(N_CORES):
        tgc = np.empty((128, 64), dtype=np.int32)
        tgc[:, :32] = tg[i * ROWS : (i + 1) * ROWS].reshape(128, 32)
        tgc[:, 32:48] = avec
        tgc[:, 48:64] = encv
        in_maps.append(
            {
                "lp": lp[i * BS : (i + 1) * BS],
                "tgc": tgc,
            }
        )
    return in_maps


def _combine(results):
    tot = 0.0
    cnt = 0.0
    for r in results:
        o = np.asarray(r["out"], dtype=np.float64)
        tot += o[:, 0].sum() + o[:, 1].sum()
        cnt += o[:, 2].sum()
    if cnt > 0:
        return np.array(PENALTY_SCALE * tot / cnt, dtype=np.float32)
    return np.array(0.0, dtype=np.float32)


def kernel(log_probs, targets, target_lengths, **_kwargs):
    from concourse.bass_utils import run_bass_kernel_spmd

    nc = _get_nc()
    in_maps = _shard_inputs(log_probs, targets)
    res = run_bass_kernel_spmd(
        nc, in_maps, list(range(N_CORES)), **_CACHE.get("run_kwargs", {})
    )
    _CACHE["last_result"] = res
    return _combine(res.results)
